# revision 63
# baseline (speedup 1.0000x reference)
"""Grouped per-sample MLP (conv1d groups=B) + GroupSwish + softmax, on 8 NeuronCores.

Data-parallel over the group/batch axis B=256: 32 groups per core,
processed as 8 quads of 4 groups packed into the 128-partition dim.

Per group g: h = W1[g] @ x[g] + b1[g]; GroupSwish; o = W2[g] @ h + b2[g];
softmax over the flattened [C*L] logits.

The kernel is HBM-stream-bound (~290 GB/s/core under 8-core load,
~14MB/core => ~48us stream floor; plus ~6us fixed framework epilogue -
a 253-semaphore teardown sweep - inside the measured window). Trace-
verified design notes:
  - x and W1 ship as fp8e4m3, swish intermediate fp16, out bf16 (cast
    host-side). End-to-end rel err ~9e-3 vs the 2e-2 gate.
  - x streams via two DMA rings whose rates ADD (sync HWDGE carries
    chunks 0-3 of each quad, gpsimd SWDGE chunks 4-6; together ~295
    GB/s). Big per-quad transfers matter: many small granules collapse
    per-ring throughput (~0.4us ring bubble per DMA), and each
    dma_start's issue-wait briefly serializes its engine to DMA
    completion pace. The last quad is split (0,4)/(4,6)/(6,7) with the
    final small piece on the gpsimd ring (less end-of-stream backlog) so
    its W1 matmuls trail the stream.
  - GroupSwish uses Tanh: Tanh and Exp share ACT function-table set 0,
    so the ACT engine never swaps tables. (Silu/Sigmoid live in other
    sets; using them costs a 1.3us ACT_TABLE_LOAD around EVERY
    activation - measured +9us end-to-end.)
  - W2 is ONE block-diagonal [128, 40] fp16 matmul per quad
    (w2c[32j+z, 10j+m] = W2[g][m,z]/1.1): the logits land compactly at
    partitions 10j+m, softmax runs on [40, L] with no pad lanes, the
    denominators come from one [40,40] block-mask matmul, and the store
    is ONE plain contiguous [40, 512] DMA per quad (alternating rings)
    instead of four strided ones - this is the main win over the
    previous revision (fewer serial ~0.7us store issues in the tail).
  - b1 is folded into the h PSUM accumulation by a K=1 matmul (lhsT =
    b1 row, rhs = ones) emitted before the W1 matmuls - it has no x
    dependency so it fills idle PE time, and it removes the (h+b1) DVE
    op from the swish: sw = (tanh(sp*h'/2)+1)*h' is just ACT + one DVE
    op, with the swish's *0.5 folded into W2 (/2.2 total).
  - All of W1 stays resident in SBUF (one DMA before the x stream);
    softplus(beta), b2 and all scale folding are host-side.
  - Software-pipelined emission: per iteration q the stream is
    [stage1(q)] [swish(q)] [W2+exp(q-1)] [totals+recip(q-2)]
    [normalize+store(q-3)] so every cross-engine dependency has a quad
    of slack before an in-order engine queue needs it.
"""

import os
import ml_dtypes
import numpy as np
from contextlib import ExitStack

import concourse.mybir as mybir
import concourse.tile as tile
from concourse import bacc
from concourse.bass_utils import run_bass_kernel_spmd

B, X, Z, C, L = 256, 784, 32, 10, 512
NCORE = 8
GPC = B // NCORE  # 32 groups per core
NQ = GPC // 4  # 8 quads per core
KC = 112  # K-chunk size (7 * 112 = 784)
NCH = 7
P = 128
F32 = mybir.dt.float32
F16 = mybir.dt.float16
F8 = mybir.dt.float8e4
BF16 = mybir.dt.bfloat16

DEFAULT_CFG = dict(
    x_bufs=5,
    w_bufs=3,
    s_bufs=4,
    h_bufs=2,
    o_bufs=2,
    x_layout="cc",  # "jp": j-split halves, 14KB descs; "cc": c-split, 2KB descs
    x_engines=("sync", "gpsimd"),
    w_engine="sync",
    out_engine="gpsimd",
    out2_engine="sync",
    const_engine="gpsimd",
)

_CACHE: dict = {}


def _eng(nc, name):
    return getattr(nc, name)


def _build(cfg=DEFAULT_CFG):
    nc = bacc.Bacc("TRN2", target_bir_lowering=False, debug=False)

    if cfg["x_layout"] == "jp":
        xq = nc.dram_tensor(
            "xq", [NQ * 2, KC, 2 * NCH * L], F8, kind="ExternalInput"
        ).ap()
    else:
        xq = nc.dram_tensor(
            "xq", [NQ, NCH, KC, 4 * L], F8, kind="ExternalInput"
        ).ap()
    w1q = nc.dram_tensor(
        "w1q", [KC, NQ * 4 * NCH * Z], F8, kind="ExternalInput"
    ).ap()
    # w2c[32j+z, 40q+10j+m] = W2[4q+j, m, z] / 1.1 (block-diagonal compact)
    w2q = nc.dram_tensor("w2q", [P, NQ * 40], F16, kind="ExternalInput").ap()
    # b1row[0, 128q+32j+z] = b1[4q+j, z]: K=1 matmul folds b1 into the
    # h PSUM accumulation, so the swish needs no separate (h+b1) DVE op
    b1row = nc.dram_tensor("b1row", [1, NQ * P], BF16, kind="ExternalInput").ap()
    onesb = nc.dram_tensor("onesb", [1, L], BF16, kind="ExternalInput").ap()
    sphq = nc.dram_tensor("sphq", [P, NQ], F32, kind="ExternalInput").ap()
    b2q = nc.dram_tensor("b2q", [P, NQ], F32, kind="ExternalInput").ap()
    # maskc[p, m] = 1 iff p//10 == m//10  (p, m < 40)
    maskb = nc.dram_tensor("maskb", [40, 40], BF16, kind="ExternalInput").ap()
    out = nc.dram_tensor("out", [GPC * C, L], BF16, kind="ExternalOutput").ap()

    with tile.TileContext(nc) as tc, ExitStack() as ctx:
        consts = ctx.enter_context(tc.tile_pool(name="consts", bufs=1))
        xpool = ctx.enter_context(tc.tile_pool(name="x", bufs=2 * cfg["x_bufs"]))
        spool = ctx.enter_context(tc.tile_pool(name="act", bufs=cfg["s_bufs"]))
        hps = ctx.enter_context(
            tc.tile_pool(name="hps", bufs=cfg["h_bufs"], space="PSUM")
        )
        ops = ctx.enter_context(
            tc.tile_pool(name="ops", bufs=cfg["o_bufs"], space="PSUM")
        )
        tps = ctx.enter_context(tc.tile_pool(name="tps", bufs=2, space="PSUM"))

        ce = _eng(nc, cfg["const_engine"])
        xes = [_eng(nc, e) for e in cfg["x_engines"]]
        we = _eng(nc, cfg["w_engine"])
        oe = _eng(nc, cfg["out_engine"])
        o2e = _eng(nc, cfg["out2_engine"])

        w1t = consts.tile([KC, NQ * 4 * NCH * Z], F8, name="w1t")
        we.dma_start(w1t[:], w1q)
        w2t = consts.tile([P, NQ * 40], F16, name="w2t")
        ce.dma_start(w2t[:], w2q)
        b1rt = consts.tile([1, NQ * P], BF16, name="b1rt")
        ce.dma_start(b1rt[:], b1row)
        onest = consts.tile([1, L], BF16, name="onest")
        ce.dma_start(onest[:], onesb)
        spht = consts.tile([P, NQ], F32, name="spht")
        ce.dma_start(spht[:], sphq)
        b2t = consts.tile([P, NQ], F32, name="b2t")
        ce.dma_start(b2t[:], b2q)
        maskt = consts.tile([40, 40], BF16, name="maskt")
        ce.dma_start(maskt[:], maskb)

        hqs, swishes, expos, esums, invcs = {}, {}, {}, {}, {}

        def w1s(q, j, c):
            k = (q * 4 + j) * NCH + c
            return w1t[:, k * Z : (k + 1) * Z]

        def stage1(q):
            """x loads (two halves), W1 matmuls for quad q. The h PSUM
            accumulation is seeded with b1 by a K=1 matmul (no x
            dependency, fills otherwise-idle PE time)."""
            hq = hps.tile([P, L], F32, tag="h", name=f"h{q}")
            hqs[q] = hq
            nc.tensor.matmul(
                hq[:],
                b1rt[:, q * P : (q + 1) * P],
                onest[:],
                start=True,
                stop=False,
                skip_group_check=True,
            )
            if cfg["x_layout"] == "jp":
                xts = []
                for h in range(2):
                    xt = xpool.tile(
                        [KC, 2 * NCH * L], F8, tag="xt", name=f"xt{q}_{h}"
                    )
                    xes[h % len(xes)].dma_start(xt[:], xq[2 * q + h])
                    xts.append(xt)
                for j in range(4):
                    xt = xts[j // 2]
                    for c in range(NCH):
                        k = (j % 2) * NCH + c
                        nc.tensor.matmul(
                            hq[32 * j : 32 * j + 32, :],
                            w1s(q, j, c),
                            xt[:, k * L : (k + 1) * L],
                            start=(c == 0),
                            stop=(c == NCH - 1),
                            tile_position=(0, 32 * j),
                        )
            else:
                if q == NQ - 1:
                    splits = [(0, 4), (4, 6), (6, 7)]
                else:
                    splits = [(0, 4), (4, 7)]
                xts = []
                for si, (c0, c1) in enumerate(splits):
                    nchunks = c1 - c0
                    xt = xpool.tile(
                        [KC, nchunks * 4 * L], F8, tag=f"xt{si}", name=f"x{q}_{si}"
                    )
                    xes[min(si, 1)].dma_start(
                        xt[:].rearrange("p (c r) -> p c r", c=nchunks),
                        xq[q, c0:c1].rearrange("c p r -> p c r"),
                    )
                    xts.append(xt)
                for c in range(NCH):
                    si = next(i for i, (c0, c1) in enumerate(splits) if c < c1)
                    xt, cc = xts[si], c - splits[si][0]
                    for j in range(4):
                        nc.tensor.matmul(
                            hq[32 * j : 32 * j + 32, :],
                            w1s(q, j, c),
                            xt[:, (cc * 4 + j) * L : (cc * 4 + j + 1) * L],
                            start=False,
                            stop=(c == NCH - 1),
                            tile_position=(0, 32 * j),
                            skip_group_check=True,
                        )

        def stage_swish(q):
            """h' = h+b1 already in PSUM; sw = (tanh(sp*h'/2)+1)*h',
            with the swish's *0.5 and /1.1 folded into W2 host-side."""
            hq = hqs.pop(q)
            t = spool.tile([P, L], F32, tag="t", name=f"t{q}")
            nc.scalar.activation(
                t[:],
                hq[:],
                mybir.ActivationFunctionType.Tanh,
                bias=0.0,
                scale=spht[:, q : q + 1],
            )
            sw = spool.tile([P, L], F16, tag="sw", name=f"sw{q}")
            nc.vector.scalar_tensor_tensor(
                sw[:],
                t[:],
                1.0,
                hq[:],
                op0=mybir.AluOpType.add,
                op1=mybir.AluOpType.mult,
            )
            swishes[q] = sw

        def stage2(q):
            """Single block-diagonal W2 matmul: compact [40, L] logits."""
            sw = swishes.pop(q)
            o = ops.tile([40, L], F32, tag="o", name=f"o{q}")
            nc.tensor.matmul(
                o[:],
                w2t[:, q * 40 : (q + 1) * 40],
                sw[:],
                start=True,
                stop=True,
            )
            expo = spool.tile([40, L], F32, tag="expo", name=f"e{q}")
            esum = spool.tile([40, 1], BF16, tag="esum", name=f"es{q}")
            with nc.allow_low_precision(reason="softmax denom, 2e-2 gate"):
                nc.scalar.activation(
                    expo[:],
                    o[:],
                    mybir.ActivationFunctionType.Exp,
                    bias=b2t[0:40, q : q + 1],
                    scale=1.0,
                    accum_out=esum[:],
                )
            expos[q] = expo
            esums[q] = esum

        def stage3a(q):
            esum = esums.pop(q)
            tot = tps.tile([40, 1], F32, tag="tot", name=f"tot{q}")
            nc.tensor.matmul(tot[:], maskt[:], esum[:], start=True, stop=True)
            invc = spool.tile([40, 1], F32, tag="invc", name=f"ic{q}")
            nc.vector.reciprocal(invc[:], tot[:])
            invcs[q] = invc

        def stage3b(q):
            invc = invcs.pop(q)
            expo = expos.pop(q)
            res = spool.tile([40, L], BF16, tag="res", name=f"r{q}")
            nc.vector.tensor_scalar_mul(res[:], expo[:], invc[:])
            e = oe if q % 2 == 0 else o2e
            e.dma_start(out[40 * q : 40 * q + 40], res[:])

        for q in range(NQ + 3):
            if q < NQ:
                stage1(q)
                stage_swish(q)
            if 1 <= q <= NQ:
                stage2(q - 1)
            if 2 <= q <= NQ + 1:
                stage3a(q - 2)
            if q >= 3:
                stage3b(q - 3)

    nc.compile()
    return nc


def _marshal(x, W1, b1, beta, W2, b2, cfg=DEFAULT_CFG):
    xg = np.asarray(x, dtype=np.float32).reshape(B, NCH, KC, L)
    w1T = np.asarray(W1, dtype=np.float32).transpose(0, 2, 1)  # [B, X, Z]
    w1g = w1T.reshape(B, NCH, KC, Z)  # (g, c, p, z)
    w2s = (np.asarray(W2, dtype=np.float32) * np.float32(1.0 / 2.2))  # [B, C, Z]
    b1f = np.asarray(b1, dtype=np.float32)  # [B, Z]
    b2f = np.asarray(b2, dtype=np.float32)  # [B, C]
    bf = np.asarray(beta, dtype=np.float32)  # [B]
    sph = np.log1p(np.exp(bf)) * np.float32(0.5)  # softplus(beta)/2

    pp = np.arange(40)
    maskb = (pp[:, None] // C == pp[None, :] // C).astype(ml_dtypes.bfloat16)

    in_maps = []
    for core in range(NCORE):
        s = slice(core * GPC, (core + 1) * GPC)
        if cfg["x_layout"] == "jp":
            xc = xg[s].reshape(NQ, 2, 2, NCH, KC, L)
            xqm = (
                xc.transpose(0, 1, 4, 2, 3, 5)
                .astype(ml_dtypes.float8_e4m3)
                .reshape(NQ * 2, KC, 2 * NCH * L)
            )
        else:
            xc = xg[s].reshape(NQ, 4, NCH, KC, L)
            xqm = (
                xc.transpose(0, 2, 3, 1, 4)
                .astype(ml_dtypes.float8_e4m3)
                .reshape(NQ, NCH, KC, 4 * L)
            )
        wc = w1g[s].reshape(NQ, 4, NCH, KC, Z)
        w1qm = (
            wc.transpose(3, 0, 1, 2, 4)
            .astype(ml_dtypes.float8_e4m3)
            .reshape(KC, NQ * 4 * NCH * Z)
        )
        w2c = w2s[s].reshape(NQ, 4, C, Z)  # (q, j, m, z)
        w2qm = np.zeros((4, Z, NQ, 4, C), np.float16)
        for j in range(4):
            w2qm[j, :, :, j, :] = w2c[:, j].transpose(2, 0, 1)  # (z, q, m)
        w2qm = w2qm.reshape(P, NQ * 40)
        b1qm = np.ascontiguousarray(
            b1f[s].reshape(NQ, 4, Z).transpose(1, 2, 0)
        ).reshape(P, NQ)
        sphqm = np.ascontiguousarray(
            np.broadcast_to(
                sph[s].reshape(NQ, 4).T[:, None, :], (4, Z, NQ)
            )
        ).reshape(P, NQ)
        b1rowm = np.ascontiguousarray(b1qm.T).reshape(1, NQ * P).astype(
            ml_dtypes.bfloat16
        )
        b2qm = np.zeros((P, NQ), np.float32)
        b2qm[0:40] = (
            b2f[s].reshape(NQ, 4, C).transpose(1, 2, 0).reshape(40, NQ)
        )
        in_maps.append(
            {
                "xq": xqm,
                "w1q": w1qm,
                "w2q": w2qm,
                "b1row": b1rowm,
                "onesb": np.ones((1, L), ml_dtypes.bfloat16),
                "sphq": sphqm,
                "b2q": b2qm,
                "maskb": maskb,
            }
        )
    return in_maps


def _run(in_maps, cfg=DEFAULT_CFG, trace=False, tmpdir=None):
    key = str(sorted(cfg.items()))
    if key not in _CACHE:
        _CACHE[key] = _build(cfg)
    return run_bass_kernel_spmd(
        _CACHE[key],
        in_maps,
        core_ids=list(range(NCORE)),
        trace=trace,
        tmpdir=tmpdir,
    )


_LAST = {}


def kernel(x, W1, b1, beta, W2, b2):
    cfg = dict(DEFAULT_CFG)
    ov = os.environ.get("KERNEL_CFG")
    if ov:
        for kv in ov.split(","):
            k, v = kv.split("=")
            cfg[k] = type(DEFAULT_CFG[k])(eval(v)) if not isinstance(
                DEFAULT_CFG[k], str
            ) else v
    in_maps = _marshal(x, W1, b1, beta, W2, b2, cfg)
    trace = bool(os.environ.get("KERNEL_TRACE"))
    r = _run(in_maps, cfg, trace=trace, tmpdir=os.environ.get("KERNEL_TRACE_DIR"))
    _LAST["results"] = r
    outs = [
        r.results[c]["out"].astype(np.float32).reshape(GPC, C * L)
        for c in range(NCORE)
    ]
    return np.concatenate(outs, axis=0)
